# revision 4
# baseline (speedup 1.0000x reference)
"""ARD-RBF kernel matrix on 8 TRN2 NeuronCores.

Math (reference):
    alpha = softmax(alpha_raw^2)            (D,)
    var   = variance_raw^2                  scalar
    sq_ij = sum_d alpha_d (x1_id - x2_jd)^2
    out   = var * exp(-0.5 * sq)            (N, M) f32

Device formulation (per core, rows of x1 sharded 8 ways):
    out_ij = exp( cross_ij - 0.5*rb_j - 0.5*ra_i + ln var )
    cross  = x1 @ (alpha * x2)^T            bf16 matmul, f32 PSUM accum
    -0.5*rb_j folded into the PSUM accumulation via a k=1 matmul
        (ones[1,128]^T @ rbneg[1,512]) so the whole epilogue is one
        ScalarE Exp activation with per-partition bias (-0.5*ra_i + ln var).

Host does only O(N*D) prep: softmax(alpha), row norms ra/rb, transposes,
bf16 casts. All O(N*M*D) matmul + O(N*M) exp + 256MB output IO on device.
"""

import math
import sys

import numpy as np

if "/opt/trn_rl_repo" not in sys.path:
    sys.path.insert(0, "/opt/trn_rl_repo")

import ml_dtypes

N, M, D = 8192, 8192, 256
NCORES = 8
NS = N // NCORES          # 1024 rows of x1 per core
P = 128                   # partitions
KT = D // P               # 2 k-tiles
NT = NS // P              # 8 i-tiles per core
JG = 2048                 # ScalarE activation group (4 PSUM banks)
NJ = 512                  # matmul moving free dim (1 PSUM bank)

_F16 = np.float16

_compiled = None  # cache (nc, ) across calls


def _build():
    import concourse.bass as bass
    import concourse.mybir as mybir

    dt = mybir.dt
    nc = bass.Bass()

    x1t = nc.declare_dram_parameter("x1t", [KT, P, NS], dt.float16, isOutput=False)
    x2t = nc.declare_dram_parameter("x2t", [KT, P, M], dt.float16, isOutput=False)
    rbn = nc.declare_dram_parameter("rbn", [1, M], dt.float16, isOutput=False)
    one = nc.declare_dram_parameter("one", [1, P], dt.float16, isOutput=False)
    bia = nc.declare_dram_parameter("bia", [P, NT], dt.float32, isOutput=False)
    out = nc.declare_dram_parameter("out", [NS, M], dt.float32, isOutput=True)

    ngi = M // JG        # 4 ScalarE groups per i-tile
    njc = JG // NJ       # 4 matmul column chunks per group
    PS_BUFS = 2
    OT_BUFS = 2
    NIN = 7              # input DMAs (bias issued last)
    exp_f = mybir.ActivationFunctionType.Exp

    with (
        nc.sbuf_tensor("x1s", [P, KT * NS], dt.float16) as x1s,
        nc.sbuf_tensor("x2s", [P, KT * M], dt.float16) as x2s,
        nc.sbuf_tensor("rbs", [1, M], dt.float16) as rbs,
        nc.sbuf_tensor("ons", [1, P], dt.float16) as ons,
        nc.sbuf_tensor("bis", [P, NT], dt.float32) as bis,
        nc.sbuf_tensor("ot0", [P, M], dt.float32) as ot0,
        nc.sbuf_tensor("ot1", [P, M], dt.float32) as ot1,
        nc.psum_tensor("ps0", [P, JG], dt.float32) as ps0,
        nc.psum_tensor("ps1", [P, JG], dt.float32) as ps1,
        nc.semaphore("din") as din,
        nc.semaphore("pes") as pes,
        nc.semaphore("acs") as acs,
        nc.semaphore("dout") as dout,
        nc.Block() as block,
    ):
        ots = [ot0, ot1]
        pss = [ps0, ps1]

        @block.sync
        def _(sync):
            for k in range(KT):
                sync.dma_start(x1s[:, k * NS:(k + 1) * NS], x1t[k]).then_inc(din, 16)
                sync.dma_start(x2s[:, k * M:(k + 1) * M], x2t[k]).then_inc(din, 16)
            sync.dma_start(rbs[:, :], rbn[:, :]).then_inc(din, 16)
            sync.dma_start(ons[:, :], one[:, :]).then_inc(din, 16)
            sync.dma_start(bis[:, :], bia[:, :]).then_inc(din, 16)
            for t in range(NT):
                sync.wait_ge(acs, (t + 1) * ngi)
                sync.dma_start(out[t * P:(t + 1) * P, :], ots[t % OT_BUFS][:, :]).then_inc(dout, 16)
            sync.wait_ge(dout, 16 * NT)

        @block.tensor
        def _(tensor):
            tensor.wait_ge(din, 16 * (NIN - 1))  # everything but bias
            for t in range(NT):
                for g in range(ngi):
                    G = t * ngi + g
                    if G >= PS_BUFS:
                        tensor.wait_ge(acs, G - PS_BUFS + 1)
                    ps = pss[G % PS_BUFS]
                    for k in range(KT):
                        for j in range(njc):
                            col = g * JG + j * NJ
                            tensor.matmul(
                                ps[:, j * NJ:(j + 1) * NJ],
                                x1s[:, k * NS + t * P: k * NS + (t + 1) * P],
                                x2s[:, k * M + col: k * M + col + NJ],
                                start=(k == 0),
                                stop=False,
                            )
                    for j in range(njc):
                        col = g * JG + j * NJ
                        mm = tensor.matmul(
                            ps[:, j * NJ:(j + 1) * NJ],
                            ons[0:1, :],
                            rbs[0:1, col:col + NJ],
                            start=False,
                            stop=True,
                        )
                    mm.then_inc(pes)

        @block.scalar
        def _(scalar):
            scalar.wait_ge(din, 16 * NIN)
            for t in range(NT):
                if t >= OT_BUFS:
                    scalar.wait_ge(dout, 16 * (t - OT_BUFS + 1))
                for g in range(ngi):
                    G = t * ngi + g
                    scalar.wait_ge(pes, G + 1)
                    scalar.activation(
                        ots[t % OT_BUFS][:, g * JG:(g + 1) * JG],
                        pss[G % PS_BUFS][:, :],
                        exp_f,
                        bias=bis[:, t:t + 1],
                        scale=1.0,
                    ).then_inc(acs)

    return nc


def _prep(x1, x2, alpha_raw, variance_raw):
    x1 = np.ascontiguousarray(np.asarray(x1, dtype=np.float32))
    x2 = np.ascontiguousarray(np.asarray(x2, dtype=np.float32))
    ar = np.asarray(alpha_raw, dtype=np.float64).reshape(-1)
    vr = np.asarray(variance_raw, dtype=np.float64).reshape(-1)

    a2 = ar * ar
    e = np.exp(a2 - a2.max())
    alpha = e / e.sum()                                   # (D,) f64
    var = float(vr[0]) ** 2
    if var > 0.0:
        logvar, post = math.log(var), None
    else:
        logvar, post = 0.0, var

    b = alpha[None, :] * x2.astype(np.float64)            # (M, D)
    x2t = np.ascontiguousarray(b.T.reshape(KT, P, M).astype(_F16))
    x1t = np.ascontiguousarray(x1.T.reshape(KT, P, N).astype(_F16))

    ra = (x1.astype(np.float64) ** 2) @ alpha             # (N,)
    rb = (x2.astype(np.float64) ** 2) @ alpha             # (M,)
    bia = (-0.5 * ra + logvar).astype(np.float32)         # (N,)
    rbn = np.ascontiguousarray((-0.5 * rb).astype(_F16).reshape(1, M))
    ones = np.ones((1, P), dtype=_F16)

    in_maps = []
    for c in range(NCORES):
        # bias laid out [P, NT]: bia2[p, t] = bias for row t*128+p of the shard
        bia2 = np.ascontiguousarray(bia[c * NS:(c + 1) * NS].reshape(NT, P).T)
        in_maps.append({
            "x1t": np.ascontiguousarray(x1t[:, :, c * NS:(c + 1) * NS]),
            "x2t": x2t,
            "rbn": rbn,
            "one": ones,
            "bia": bia2,
        })
    return in_maps, post


def _run(in_maps, trace=False):
    global _compiled
    from concourse.bass_utils import run_bass_kernel_spmd

    if _compiled is None:
        _compiled = _build()
    return run_bass_kernel_spmd(
        _compiled, in_maps, core_ids=list(range(NCORES)), trace=trace
    )


def kernel(x1, x2, alpha_raw, variance_raw):
    in_maps, post = _prep(x1, x2, alpha_raw, variance_raw)
    res = _run(in_maps)
    full = np.concatenate([res.results[c]["out"] for c in range(NCORES)], axis=0)
    if post is not None:
        full = (full * post).astype(np.float32)
    return full
